# revision 4
# baseline (speedup 1.0000x reference)
"""ARMA GNN Trainium2 kernel (8 NeuronCores).

kernel(**inputs) takes the FULL unsharded inputs and returns the FULL
[N, NCLS] float32 log-softmax output. See gnn_builder.py for design notes.
"""
import time
import numpy as np

import gnn_builder as G
from concourse.bass_utils import run_bass_kernel_spmd

LAST_EXEC_NS = None
TRACE = True


def kernel(**inputs) -> np.ndarray:
    global LAST_EXEC_NS
    if TRACE:
        try:
            import ntff_hook
            ntff_hook.install()
        except Exception as e:
            print(f"  ntff hook install failed: {e}")
    p = G.P()
    t0 = time.time()
    nc, in_maps = G.prepare(p, inputs)
    t1 = time.time()
    print(f"  prepare(build+preproc): {t1-t0:.1f}s")
    res = run_bass_kernel_spmd(nc, in_maps, core_ids=list(range(p.NCORE)),
                               trace=TRACE)
    t2 = time.time()
    print(f"  compile+run: {t2-t1:.1f}s")
    LAST_EXEC_NS = res.exec_time_ns
    out = np.concatenate([res.results[c]["out"] for c in range(p.NCORE)], axis=0)
    return out.astype(np.float32)
